# revision 16
# baseline (speedup 1.0000x reference)
"""BlendShapes model kernel for 8 Trainium2 NeuronCores (warm-PE design).

Computation (reference):
    pose_repr = pose[:, 1:].reshape(B, 23, 9) - eye      # (B, J, 9)
    per-joint MLP 9 -> 18 -> 32 -> 8 (ReLU between)      # coff (B, J, 8)
    basis_full = basis[:, None] * mask[:, :, None, None]  # (V, J, 8, 3)
    res = einsum('bjk,vjkc->bvc', coff, basis_full)       # (B, V, 3)

Mapping (per core; vertices sharded 8 ways, V=6890 padded to 8*864):
  - Host precomputes bfm = basis*mask*2^13 (f16, rows (j,k), cols (v,c)) and
    folds the eye-subtraction into the L1 bias (b1' = b1 - e @ W1), so the
    device does only matmuls + epilogues + the output store.
  - The PE's HAM clock gate throttles matmuls to 1.2 GHz until ~6us of
    sustained activity, then 2.4 GHz. Warm-up matmuls run during the input
    DMAs; the kernel avoids PE-idle gaps afterwards.
  - Input DMAs: w + pose on the sync queue (MLP critical path, uncontended);
    bias + bfm on the gpsimd queue, with bfm fenced behind the last pose
    chunk (a dummy read) so its 950KB doesn't starve the pose transfers.
  - MLP joint chunks of 4 (6 chunks):
      L1 (K=36, M=72):  chunk pairs row-tiled at PE rows 0 / 64 -> 2x
      L2 (K=72, M=128): plain matmuls
      L3 (K=128, M=32): 4 chunks col-tiled into one PSUM tile -> the
        coefficients land directly in coffT layout (no merge DMAs)
    PSUM tiles hold both 512-wide batch halves ([*, 1024]) so each chunk
    needs ONE epilogue op (bigger ACT/DVE ops amortize fixed overhead).
  - Main GEMM out[b, (v,c)] = coffT.T @ bfm, K=184 split 128+56, per b-tile
    stationary reuse across six 512-wide N-tiles (LDWEIGHTS stays hidden),
    three [128,1024] PSUM tiles, per-tile evacuation + store (pipelined).
  - Output stored f16 (descale 2^-13 folded into the evacuation); host
    converts to f32.
"""

import numpy as np

N_VERT, N_JOINT, BPJ, BATCH = 6890, 23, 8, 1024
VPAD = 6912  # 8 * 864
VC = VPAD // 8  # 864 vertices per core
VC3 = VC * 3  # 2592
NB = BATCH // 128  # 8 b-tiles
NT_BOUNDS = [0, 512, 1024, 1536, 2048, 2560, 2592]
PAIR_BOUNDS = [0, 1024, 2048, 2592]

CHUNKS = [(0, 4), (4, 8), (8, 12), (12, 16), (16, 20), (20, 23)]
NCH = len(CHUNKS)


def _offsets(mpj):
    offs, col = [], 0
    for js, je in CHUNKS:
        offs.append(col)
        col += (je - js) * mpj
    return offs, col


W1_OFF, W1_TOT = _offsets(18)  # 414
W2_OFF, W2_TOT = _offsets(32)  # 736
W3_OFF, W3_TOT = _offsets(8)   # 184
W2_OFF = [W1_TOT + o for o in W2_OFF]
W3_OFF = [W1_TOT + W2_TOT + o for o in W3_OFF]
W_COLS = W1_TOT + W2_TOT + W3_TOT  # 1334

# bias_all columns: [0:6] L1 bias (eye-folded), [6:12] L2 bias,
# [12] L3 bias stacked for coffT_a (128 rows), [13] same for coffT_b (56).
BIAS_COLS = 14
BSCALE = 8192.0  # 2**13, exact in f16/f32
DESCALE = 1.0 / 8192.0
N_WARMUP = 11  # warm-up matmuls (N=512) before the MLP

_CACHED = {}


def _build_nc():
    import concourse.tile as tile
    from concourse import bacc, mybir
    from contextlib import ExitStack

    dt = mybir.dt
    f32, f16 = dt.float32, dt.float16
    AF = mybir.ActivationFunctionType
    ALU = mybir.AluOpType

    nc = bacc.Bacc(None, target_bir_lowering=False)

    # mega input: [128, W_COLS + 3*1024] f16 = w_all columns followed by the
    # three pose-pair tile images (chunk 2p at rows 0.., 2p+1 at rows 64..).
    MEGA_COLS = W_COLS + 3 * BATCH
    mega_d = nc.dram_tensor("mega", [128, MEGA_COLS], f16, kind="ExternalInput")
    bfm_a_d = nc.dram_tensor("bfm_a", [128, VC3], f16, kind="ExternalInput")
    # bfm_b duplicated at rows 0-55 and 64-119 (the B-pass runs two b-tiles
    # concurrently in PE row groups 0 and 64).
    bfm_b_d = nc.dram_tensor("bfm_b", [128, VC3], f16, kind="ExternalInput")
    bias_all = nc.dram_tensor("bias_all", [128, BIAS_COLS], f32, kind="ExternalInput")
    res = nc.dram_tensor("res", [BATCH, VC3], f16, kind="ExternalOutput")

    with ExitStack() as ctx:
        tc = ctx.enter_context(tile.TileContext(nc))
        const = ctx.enter_context(tc.tile_pool(name="const", bufs=1))
        work = ctx.enter_context(tc.tile_pool(name="work", bufs=1))
        outp = ctx.enter_context(tc.tile_pool(name="outp", bufs=2))
        psum = ctx.enter_context(tc.tile_pool(name="psum", bufs=4, space="PSUM"))

        # warm-up source: memset on DVE (its queue opens early).
        warm = work.tile([128, 512], f16, tag="warm")
        nc.vector.memset(warm[:], 0.0)

        # ---- input DMAs. ONE mega DMA for w + pose (128 big packets beat
        # many small per-chunk jobs: each DMA job pays ~1us latency and the
        # engines serialize small packets at ~100-500ns apiece).
        mega = const.tile([128, MEGA_COLS], f16, tag="mega")
        nc.sync.dma_start(out=mega[:], in_=mega_d[:, :])
        w_sb = mega[:, 0:W_COLS]
        pose_p = [
            mega[:, W_COLS + p * BATCH : W_COLS + (p + 1) * BATCH] for p in range(3)
        ]

        bias_sb = const.tile([128, BIAS_COLS], f32, tag="bias")
        nc.gpsimd.dma_start(out=bias_sb[:], in_=bias_all[:, :])
        # fence: write a byte into each bfm tile that depends on the pose
        # data, so the bfm DMAs (WAW on those tiles) cannot start until the
        # mega DMA completed -- keeps 950KB of bfm traffic off the MLP
        # critical path. (A standalone read is NOT enough: the scheduler
        # orders by data deps, not program order.)
        bfm_a = work.tile([128, VC3], f16, tag="bfm_a")
        bfm_b = work.tile([128, VC3], f16, tag="bfm_b")
        nc.gpsimd.tensor_scalar(
            out=bfm_a[64:65, 0:1], in0=pose_p[2][64:65, 1023:1024], scalar1=1.0,
            scalar2=None, op0=ALU.mult,
        )
        nc.gpsimd.tensor_scalar(
            out=bfm_b[32:33, 0:1], in0=pose_p[2][64:65, 1022:1023], scalar1=1.0,
            scalar2=None, op0=ALU.mult,
        )
        nc.gpsimd.dma_start(out=bfm_a[:], in_=bfm_a_d[:, :])
        nc.gpsimd.dma_start(out=bfm_b[:], in_=bfm_b_d[:, :])

        # ---- PE warm-up.
        wps = psum.tile([128, 1024], f32, tag="ps", name="warm_ps")
        for i in range(N_WARMUP):
            nc.tensor.matmul(
                wps[:, 0:512], lhsT=warm[:, 0:128], rhs=warm[:],
                start=True, stop=True,
            )

        # ---- MLP ----
        ep_ctr = [0]

        def epilogue(dst, src, bias_ap, relu, scale=None):
            # alternate ACT / DVE (GPSIMD cannot read PSUM; ACT's Copy
            # can't take an AP bias, so bias-only epilogues go to DVE).
            e = ep_ctr[0] % 2
            ep_ctr[0] += 1
            if e == 0 and scale is not None:
                nc.scalar.activation(dst, src, AF.Copy, scale=scale)
            elif e == 0 and relu:
                nc.scalar.activation(dst, src, AF.Relu, bias=bias_ap)
            elif scale is not None:
                nc.vector.tensor_scalar(
                    out=dst, in0=src, scalar1=scale, scalar2=None, op0=ALU.mult
                )
            elif relu:
                nc.vector.tensor_scalar(
                    out=dst, in0=src, scalar1=bias_ap, scalar2=0.0,
                    op0=ALU.add, op1=ALU.max,
                )
            else:
                nc.vector.tensor_scalar(
                    out=dst, in0=src, scalar1=bias_ap, scalar2=None, op0=ALU.add
                )

        h1 = {}
        h2 = {}
        HALVES = (slice(0, 512), slice(512, 1024))

        # L1: row-tiled chunk pairs; one [*,1024] PSUM tile per chunk holds
        # both batch halves -> one epilogue per chunk.
        for p in range(3):
            c0, c1 = 2 * p, 2 * p + 1
            K0, M0 = 9 * (CHUNKS[c0][1] - CHUNKS[c0][0]), 18 * (CHUNKS[c0][1] - CHUNKS[c0][0])
            K1, M1 = 9 * (CHUNKS[c1][1] - CHUNKS[c1][0]), 18 * (CHUNKS[c1][1] - CHUNKS[c1][0])
            h1[c0] = work.tile([M0, BATCH], f16, tag=f"h1_{c0}", name=f"h1_{c0}")
            h1[c1] = work.tile([M1, BATCH], f16, tag=f"h1_{c1}", name=f"h1_{c1}")
            ps0 = psum.tile([128, 1024], f32, tag="ps", name=f"ps1_{c0}")
            ps1 = psum.tile([128, 1024], f32, tag="ps", name=f"ps1_{c1}")
            for h, hs in enumerate(HALVES):
                nc.tensor.matmul(
                    ps0[0:M0, hs], lhsT=w_sb[0:K0, W1_OFF[c0] : W1_OFF[c0] + M0],
                    rhs=pose_p[p][0:K0, hs], start=True, stop=True,
                    tile_position=(0, 0),
                )
                nc.tensor.matmul(
                    ps1[0:M1, hs], lhsT=w_sb[64 : 64 + K1, W1_OFF[c1] : W1_OFF[c1] + M1],
                    rhs=pose_p[p][64 : 64 + K1, hs], start=True, stop=True,
                    tile_position=(64, 0),
                )
            epilogue(h1[c0][:, :], ps0[0:M0, :], bias_sb[0:M0, c0 : c0 + 1], True)
            epilogue(h1[c1][:, :], ps1[0:M1, :], bias_sb[0:M1, c1 : c1 + 1], True)

        # L2: plain per-chunk matmuls, both halves into one PSUM tile.
        for c, (js, je) in enumerate(CHUNKS):
            nj = je - js
            K, M = 18 * nj, 32 * nj
            h2[c] = work.tile([M, BATCH], f16, tag=f"h2_{c}", name=f"h2_{c}")
            ps = psum.tile([128, 1024], f32, tag="ps", name=f"ps2_{c}")
            for h, hs in enumerate(HALVES):
                nc.tensor.matmul(
                    ps[0:M, hs], lhsT=w_sb[0:K, W2_OFF[c] : W2_OFF[c] + M],
                    rhs=h1[c][:, hs], start=True, stop=True,
                )
            epilogue(h2[c][:, :], ps[0:M, :], bias_sb[0:M, 6 + c : 7 + c], True)

        # L3: col-tiled into coffT layout; one PSUM tile per group covers
        # both halves. Group A: chunks 0-3 -> coffT_a partitions 32c..;
        # group B: chunks 4,5 -> coffT_b partitions 0-55.
        coffT_a = work.tile([128, BATCH], f16, tag="coffT_a")
        coffT_b = work.tile([128, BATCH], f16, tag="coffT_b")
        psA = psum.tile([128, 1024], f32, tag="ps", name="ps3a")
        for h, hs in enumerate(HALVES):
            for c in range(4):
                nc.tensor.matmul(
                    psA[32 * c : 32 * c + 32, hs],
                    lhsT=w_sb[0:128, W3_OFF[c] : W3_OFF[c] + 32],
                    rhs=h2[c][:, hs], start=True, stop=True,
                    tile_position=(0, 32 * c),
                )
        epilogue(coffT_a[:, :], psA[:, :], bias_sb[0:128, 12:13], False)
        # Group B lands twice (col positions 0/32 and 64/96) so the main
        # B-pass can run two b-tiles concurrently in PE row groups 0 / 64.
        psB = psum.tile([128, 1024], f32, tag="ps", name="ps3b")
        for h, hs in enumerate(HALVES):
            for r in (0, 64):
                nc.tensor.matmul(
                    psB[r : r + 32, hs], lhsT=w_sb[0:128, W3_OFF[4] : W3_OFF[4] + 32],
                    rhs=h2[4][:, hs], start=True, stop=True, tile_position=(0, r),
                )
                nc.tensor.matmul(
                    psB[r + 32 : r + 56, hs], lhsT=w_sb[0:96, W3_OFF[5] : W3_OFF[5] + 24],
                    rhs=h2[5][:, hs], start=True, stop=True, tile_position=(0, r + 32),
                )
        epilogue(coffT_b[0:56, :], psB[0:56, :], bias_sb[0:56, 13:14], False)
        epilogue(coffT_b[64:120, :], psB[64:120, :], bias_sb[64:120, 13:14], False)

        # ---- main GEMM, b-tiles in pairs: per 1024-wide N-chunk, A-passes
        # (K=128) for both b-tiles, then the two K=56 B-passes CONCURRENTLY
        # in PE row groups 0 / 64, then per-chunk evacuation (2^-13 descale,
        # f16) on ACT (tile i) and DVE (tile j) in parallel.
        for p in range(NB // 2):
            bti, btj = 2 * p, 2 * p + 1
            bsl_i = slice(bti * 128, bti * 128 + 128)
            bsl_j = slice(btj * 128, btj * 128 + 128)
            os_i = outp.tile([128, VC3], f16, tag="ostrip", name=f"ostrip_{bti}")
            os_j = outp.tile([128, VC3], f16, tag="ostrip", name=f"ostrip_{btj}")
            for g in range(3):
                g0, g1 = PAIR_BOUNDS[g], PAIR_BOUNDS[g + 1]
                ti = psum.tile([128, 1024], f32, tag="ps", name=f"psm_{p}_{g}_i")
                tj = psum.tile([128, 1024], f32, tag="ps", name=f"psm_{p}_{g}_j")
                subs = [
                    (slice(n0 - g0, n1 - g0), slice(n0, n1))
                    for n0, n1 in zip(NT_BOUNDS, NT_BOUNDS[1:])
                    if g0 <= n0 < g1
                ]
                for ps, bsl in ((ti, bsl_i), (tj, bsl_j)):
                    for ssl, nsl in subs:
                        nc.tensor.matmul(
                            ps[:, ssl], lhsT=coffT_a[:, bsl], rhs=bfm_a[:, nsl],
                            start=True, stop=False,
                        )
                for ssl, nsl in subs:
                    nc.tensor.matmul(
                        ti[:, ssl], lhsT=coffT_b[0:56, bsl_i],
                        rhs=bfm_b[0:56, nsl], start=False, stop=True,
                        tile_position=(0, 0),
                    )
                    nc.tensor.matmul(
                        tj[:, ssl], lhsT=coffT_b[64:120, bsl_j],
                        rhs=bfm_b[64:120, nsl], start=False, stop=True,
                        tile_position=(64, 0),
                    )
                nc.scalar.activation(
                    os_i[:, g0:g1], ti[:, 0 : g1 - g0], AF.Copy, scale=DESCALE
                )
                nc.vector.tensor_scalar(
                    out=os_j[:, g0:g1], in0=tj[:, 0 : g1 - g0], scalar1=DESCALE,
                    scalar2=None, op0=ALU.mult,
                )
                if p == NB // 2 - 1:
                    # last pair: store per chunk so the final transfer is small
                    nc.sync.dma_start(out=res[bsl_i, g0:g1], in_=os_i[:, g0:g1])
                    nc.sync.dma_start(out=res[bsl_j, g0:g1], in_=os_j[:, g0:g1])
            if p < NB // 2 - 1:
                # full-row stores: 5184B HBM segments move faster than the
                # 2048B segments of column-piece stores
                nc.sync.dma_start(out=res[bsl_i, :], in_=os_i[:])
                nc.sync.dma_start(out=res[bsl_j, :], in_=os_j[:])

    nc.finalize()
    return nc


def _pack_host(pose, basis, mask, w1, b1, w2, b2, w3, b3):
    pose_t = pose[:, 1:].reshape(BATCH, 207).T.astype(np.float16)  # [207, B]
    pose_mega = np.zeros((128, 3 * BATCH), np.float16)
    for c, (js, je) in enumerate(CHUNKS):
        K = 9 * (je - js)
        p, hi = divmod(c, 2)
        r0 = 64 if hi else 0
        pose_mega[r0 : r0 + K, p * BATCH : (p + 1) * BATCH] = (
            pose_t[9 * js : 9 * js + K]
        )

    # bfm rows (j, k) scaled by 2^13, cols (v, c) padded to VPAD.
    bfm = np.zeros((N_JOINT * BPJ, VPAD * 3), np.float16)
    prod = (basis[:, None, :, :] * mask[:, :, None, None] * BSCALE)  # (V, J, K, 3)
    bfm[:, : N_VERT * 3] = (
        prod.transpose(1, 2, 0, 3).reshape(N_JOINT * BPJ, N_VERT * 3)
    ).astype(np.float16)

    w_all = np.zeros((128, W_COLS), np.float16)
    bias_all = np.zeros((128, BIAS_COLS), np.float32)
    eye9 = np.eye(3, dtype=np.float64).reshape(-1)
    for c, ((js, je), o1, o2, o3) in enumerate(zip(CHUNKS, W1_OFF, W2_OFF, W3_OFF)):
        r1 = 64 if c % 2 else 0  # odd chunks' W1 blocks live at PE rows 64+
        for t, j in enumerate(range(js, je)):
            w_all[r1 + t * 9 : r1 + (t + 1) * 9, o1 + t * 18 : o1 + (t + 1) * 18] = w1[j]
            w_all[t * 18 : (t + 1) * 18, o2 + t * 32 : o2 + (t + 1) * 32] = w2[j]
            w_all[t * 32 : (t + 1) * 32, o3 + t * 8 : o3 + (t + 1) * 8] = w3[j]
    b1f = b1.astype(np.float64) - np.einsum("i,jio->jo", eye9, w1.astype(np.float64))
    for c, (js, je) in enumerate(CHUNKS):
        nj = je - js
        bias_all[0 : 18 * nj, c] = b1f[js:je].reshape(-1).astype(np.float32)
        bias_all[0 : 32 * nj, 6 + c] = b2[js:je].reshape(-1)
    bias_all[0:128, 12] = b3[0:16].reshape(-1)  # chunks 0-3 stacked (4*32)
    bias_all[0:56, 13] = b3[16:23].reshape(-1)  # chunks 4,5 stacked (32+24)
    bias_all[64:120, 13] = b3[16:23].reshape(-1)  # dup for PE row group 64

    mega = np.concatenate([w_all, pose_mega], axis=1)
    return mega, bfm, w_all, bias_all


def _dup_b(x):
    out = np.zeros((128, x.shape[1]), np.float16)
    out[0:56] = x
    out[64:120] = x
    return np.ascontiguousarray(out)


def _in_maps(pose, basis, mask, w1, b1, w2, b2, w3, b3):
    mega, bfm, w_all, bias_all = _pack_host(
        np.asarray(pose, np.float32),
        np.asarray(basis, np.float32),
        np.asarray(mask, np.float32),
        np.asarray(w1, np.float32),
        np.asarray(b1, np.float32),
        np.asarray(w2, np.float32),
        np.asarray(b2, np.float32),
        np.asarray(w3, np.float32),
        np.asarray(b3, np.float32),
    )
    maps = []
    for i in range(8):
        c0 = i * VC3
        maps.append(
            {
                "mega": mega,
                "bfm_a": np.ascontiguousarray(bfm[0:128, c0 : c0 + VC3]),
                "bfm_b": _dup_b(bfm[128:184, c0 : c0 + VC3]),
                "bias_all": bias_all,
            }
        )
    return maps


def kernel(pose, basis, mask, w1, b1, w2, b2, w3, b3):
    from concourse.bass_utils import run_bass_kernel_spmd

    if "nc" not in _CACHED:
        _CACHED["nc"] = _build_nc()
    nc = _CACHED["nc"]

    maps = _in_maps(pose, basis, mask, w1, b1, w2, b2, w3, b3)
    r = run_bass_kernel_spmd(nc, maps, core_ids=list(range(8)))
    out = np.concatenate(
        [m["res"].astype(np.float32).reshape(BATCH, VC, 3) for m in r.results],
        axis=1,
    )
    return np.ascontiguousarray(out[:, :N_VERT, :])


# revision 17
# speedup vs baseline: 1.2579x; 1.2579x over previous
"""BlendShapes model kernel for 8 Trainium2 NeuronCores (warm-PE design).

Computation (reference):
    pose_repr = pose[:, 1:].reshape(B, 23, 9) - eye      # (B, J, 9)
    per-joint MLP 9 -> 18 -> 32 -> 8 (ReLU between)      # coff (B, J, 8)
    basis_full = basis[:, None] * mask[:, :, None, None]  # (V, J, 8, 3)
    res = einsum('bjk,vjkc->bvc', coff, basis_full)       # (B, V, 3)

Mapping (per core; vertices sharded 8 ways, V=6890 padded to 8*864):
  - Host precomputes bfm = basis*mask*2^13 (f16, rows (j,k), cols (v,c)) and
    folds the eye-subtraction into the L1 bias (b1' = b1 - e @ W1), so the
    device does only matmuls + epilogues + the output store.
  - The PE's HAM clock gate throttles matmuls to 1.2 GHz until ~6us of
    sustained activity, then 2.4 GHz. Warm-up matmuls run during the input
    DMAs; the kernel avoids PE-idle gaps afterwards.
  - Input DMAs: w + pose on the sync queue (MLP critical path, uncontended);
    bias + bfm on the gpsimd queue, with bfm fenced behind the last pose
    chunk (a dummy read) so its 950KB doesn't starve the pose transfers.
  - MLP joint chunks of 4 (6 chunks):
      L1 (K=36, M=72):  chunk pairs row-tiled at PE rows 0 / 64 -> 2x
      L2 (K=72, M=128): plain matmuls
      L3 (K=128, M=32): 4 chunks col-tiled into one PSUM tile -> the
        coefficients land directly in coffT layout (no merge DMAs)
    PSUM tiles hold both 512-wide batch halves ([*, 1024]) so each chunk
    needs ONE epilogue op (bigger ACT/DVE ops amortize fixed overhead).
  - Main GEMM out[b, (v,c)] = coffT.T @ bfm, K=184 split 128+56, per b-tile
    stationary reuse across six 512-wide N-tiles (LDWEIGHTS stays hidden),
    three [128,1024] PSUM tiles, per-tile evacuation + store (pipelined).
  - Output stored f16 (descale 2^-13 folded into the evacuation); host
    converts to f32.
"""

import numpy as np

N_VERT, N_JOINT, BPJ, BATCH = 6890, 23, 8, 1024
VPAD = 6912  # 8 * 864
VC = VPAD // 8  # 864 vertices per core
VC3 = VC * 3  # 2592
NB = BATCH // 128  # 8 b-tiles
NT_BOUNDS = [0, 512, 1024, 1536, 2048, 2560, 2592]
PAIR_BOUNDS = [0, 1024, 2048, 2592]

CHUNKS = [(0, 4), (4, 8), (8, 12), (12, 16), (16, 20), (20, 23)]
NCH = len(CHUNKS)


def _offsets(mpj):
    offs, col = [], 0
    for js, je in CHUNKS:
        offs.append(col)
        col += (je - js) * mpj
    return offs, col


W1_OFF, W1_TOT = _offsets(18)  # 414
W2_OFF, W2_TOT = _offsets(32)  # 736
W3_OFF, W3_TOT = _offsets(8)   # 184
W2_OFF = [W1_TOT + o for o in W2_OFF]
W3_OFF = [W1_TOT + W2_TOT + o for o in W3_OFF]
W_COLS = W1_TOT + W2_TOT + W3_TOT  # 1334

# bias_all columns: [0:6] L1 bias (eye-folded), [6:12] L2 bias,
# [12] L3 bias stacked for coffT_a (128 rows), [13] same for coffT_b (56).
BIAS_COLS = 14
BSCALE = 8192.0  # 2**13, exact in f16/f32
DESCALE = 1.0 / 8192.0
N_WARMUP = 11  # warm-up matmuls (N=512) before the MLP

_CACHED = {}


def _build_nc():
    import concourse.tile as tile
    from concourse import bacc, mybir
    from contextlib import ExitStack

    dt = mybir.dt
    f32, f16 = dt.float32, dt.float16
    AF = mybir.ActivationFunctionType
    ALU = mybir.AluOpType

    nc = bacc.Bacc(None, target_bir_lowering=False)

    # mega input: [128, W_COLS + 3*1024] f16 = w_all columns followed by the
    # three pose-pair tile images (chunk 2p at rows 0.., 2p+1 at rows 64..).
    MEGA_COLS = W_COLS + 3 * BATCH
    mega_d = nc.dram_tensor("mega", [128, MEGA_COLS], f16, kind="ExternalInput")
    bfm_a_d = nc.dram_tensor("bfm_a", [128, VC3], f16, kind="ExternalInput")
    # bfm_b duplicated at rows 0-55 and 64-119 (the B-pass runs two b-tiles
    # concurrently in PE row groups 0 and 64).
    bfm_b_d = nc.dram_tensor("bfm_b", [128, VC3], f16, kind="ExternalInput")
    bias_all = nc.dram_tensor("bias_all", [128, BIAS_COLS], f32, kind="ExternalInput")
    res = nc.dram_tensor("res", [BATCH, VC3], f16, kind="ExternalOutput")

    with ExitStack() as ctx:
        tc = ctx.enter_context(tile.TileContext(nc))
        const = ctx.enter_context(tc.tile_pool(name="const", bufs=1))
        work = ctx.enter_context(tc.tile_pool(name="work", bufs=1))
        outp = ctx.enter_context(tc.tile_pool(name="outp", bufs=4))
        psum = ctx.enter_context(tc.tile_pool(name="psum", bufs=4, space="PSUM"))

        # warm-up source: memset on DVE (its queue opens early).
        warm = work.tile([128, 512], f16, tag="warm")
        nc.vector.memset(warm[:], 0.0)

        # ---- input DMAs. ONE mega DMA for w + pose (128 big packets beat
        # many small per-chunk jobs: each DMA job pays ~1us latency and the
        # engines serialize small packets at ~100-500ns apiece).
        mega = const.tile([128, MEGA_COLS], f16, tag="mega")
        nc.sync.dma_start(out=mega[:], in_=mega_d[:, :])
        w_sb = mega[:, 0:W_COLS]
        pose_p = [
            mega[:, W_COLS + p * BATCH : W_COLS + (p + 1) * BATCH] for p in range(3)
        ]

        bias_sb = const.tile([128, BIAS_COLS], f32, tag="bias")
        nc.gpsimd.dma_start(out=bias_sb[:], in_=bias_all[:, :])
        # fence: write a byte into each bfm tile that depends on the pose
        # data, so the bfm DMAs (WAW on those tiles) cannot start until the
        # mega DMA completed -- keeps 950KB of bfm traffic off the MLP
        # critical path. (A standalone read is NOT enough: the scheduler
        # orders by data deps, not program order.)
        bfm_a = work.tile([128, VC3], f16, tag="bfm_a")
        bfm_b = work.tile([128, VC3], f16, tag="bfm_b")
        nc.gpsimd.tensor_scalar(
            out=bfm_a[64:65, 0:1], in0=pose_p[2][64:65, 1023:1024], scalar1=1.0,
            scalar2=None, op0=ALU.mult,
        )
        nc.gpsimd.tensor_scalar(
            out=bfm_b[32:33, 0:1], in0=pose_p[2][64:65, 1022:1023], scalar1=1.0,
            scalar2=None, op0=ALU.mult,
        )
        nc.gpsimd.dma_start(out=bfm_a[:], in_=bfm_a_d[:, :])
        nc.gpsimd.dma_start(out=bfm_b[:], in_=bfm_b_d[:, :])

        # ---- PE warm-up.
        wps = psum.tile([128, 1024], f32, tag="ps", name="warm_ps")
        for i in range(N_WARMUP):
            nc.tensor.matmul(
                wps[:, 0:512], lhsT=warm[:, 0:128], rhs=warm[:],
                start=True, stop=True,
            )

        # ---- MLP ----
        ep_ctr = [0]

        def epilogue(dst, src, bias_ap, relu, scale=None):
            # alternate ACT / DVE (GPSIMD cannot read PSUM; ACT's Copy
            # can't take an AP bias, so bias-only epilogues go to DVE).
            e = ep_ctr[0] % 2
            ep_ctr[0] += 1
            if e == 0 and scale is not None:
                nc.scalar.activation(dst, src, AF.Copy, scale=scale)
            elif e == 0 and relu:
                nc.scalar.activation(dst, src, AF.Relu, bias=bias_ap)
            elif scale is not None:
                nc.vector.tensor_scalar(
                    out=dst, in0=src, scalar1=scale, scalar2=None, op0=ALU.mult
                )
            elif relu:
                nc.vector.tensor_scalar(
                    out=dst, in0=src, scalar1=bias_ap, scalar2=0.0,
                    op0=ALU.add, op1=ALU.max,
                )
            else:
                nc.vector.tensor_scalar(
                    out=dst, in0=src, scalar1=bias_ap, scalar2=None, op0=ALU.add
                )

        h1 = {}
        h2 = {}
        HALVES = (slice(0, 512), slice(512, 1024))

        # L1: row-tiled chunk pairs; one [*,1024] PSUM tile per chunk holds
        # both batch halves -> one epilogue per chunk.
        for p in range(3):
            c0, c1 = 2 * p, 2 * p + 1
            K0, M0 = 9 * (CHUNKS[c0][1] - CHUNKS[c0][0]), 18 * (CHUNKS[c0][1] - CHUNKS[c0][0])
            K1, M1 = 9 * (CHUNKS[c1][1] - CHUNKS[c1][0]), 18 * (CHUNKS[c1][1] - CHUNKS[c1][0])
            h1[c0] = work.tile([M0, BATCH], f16, tag=f"h1_{c0}", name=f"h1_{c0}")
            h1[c1] = work.tile([M1, BATCH], f16, tag=f"h1_{c1}", name=f"h1_{c1}")
            ps0 = psum.tile([128, 1024], f32, tag="ps", name=f"ps1_{c0}")
            ps1 = psum.tile([128, 1024], f32, tag="ps", name=f"ps1_{c1}")
            for h, hs in enumerate(HALVES):
                nc.tensor.matmul(
                    ps0[0:M0, hs], lhsT=w_sb[0:K0, W1_OFF[c0] : W1_OFF[c0] + M0],
                    rhs=pose_p[p][0:K0, hs], start=True, stop=True,
                    tile_position=(0, 0),
                )
                nc.tensor.matmul(
                    ps1[0:M1, hs], lhsT=w_sb[64 : 64 + K1, W1_OFF[c1] : W1_OFF[c1] + M1],
                    rhs=pose_p[p][64 : 64 + K1, hs], start=True, stop=True,
                    tile_position=(64, 0),
                )
            epilogue(h1[c0][:, :], ps0[0:M0, :], bias_sb[0:M0, c0 : c0 + 1], True)
            epilogue(h1[c1][:, :], ps1[0:M1, :], bias_sb[0:M1, c1 : c1 + 1], True)

        # L2: plain per-chunk matmuls, both halves into one PSUM tile.
        for c, (js, je) in enumerate(CHUNKS):
            nj = je - js
            K, M = 18 * nj, 32 * nj
            h2[c] = work.tile([M, BATCH], f16, tag=f"h2_{c}", name=f"h2_{c}")
            ps = psum.tile([128, 1024], f32, tag="ps", name=f"ps2_{c}")
            for h, hs in enumerate(HALVES):
                nc.tensor.matmul(
                    ps[0:M, hs], lhsT=w_sb[0:K, W2_OFF[c] : W2_OFF[c] + M],
                    rhs=h1[c][:, hs], start=True, stop=True,
                )
            epilogue(h2[c][:, :], ps[0:M, :], bias_sb[0:M, 6 + c : 7 + c], True)

        # L3: col-tiled into coffT layout; one PSUM tile per group covers
        # both halves. Group A: chunks 0-3 -> coffT_a partitions 32c..;
        # group B: chunks 4,5 -> coffT_b partitions 0-55.
        coffT_a = work.tile([128, BATCH], f16, tag="coffT_a")
        coffT_b = work.tile([128, BATCH], f16, tag="coffT_b")
        psA = psum.tile([128, 1024], f32, tag="ps", name="ps3a")
        for h, hs in enumerate(HALVES):
            for c in range(4):
                nc.tensor.matmul(
                    psA[32 * c : 32 * c + 32, hs],
                    lhsT=w_sb[0:128, W3_OFF[c] : W3_OFF[c] + 32],
                    rhs=h2[c][:, hs], start=True, stop=True,
                    tile_position=(0, 32 * c),
                )
        epilogue(coffT_a[:, :], psA[:, :], bias_sb[0:128, 12:13], False)
        # Group B lands twice (col positions 0/32 and 64/96) so the main
        # B-pass can run two b-tiles concurrently in PE row groups 0 / 64.
        psB = psum.tile([128, 1024], f32, tag="ps", name="ps3b")
        for h, hs in enumerate(HALVES):
            for r in (0, 64):
                nc.tensor.matmul(
                    psB[r : r + 32, hs], lhsT=w_sb[0:128, W3_OFF[4] : W3_OFF[4] + 32],
                    rhs=h2[4][:, hs], start=True, stop=True, tile_position=(0, r),
                )
                nc.tensor.matmul(
                    psB[r + 32 : r + 56, hs], lhsT=w_sb[0:96, W3_OFF[5] : W3_OFF[5] + 24],
                    rhs=h2[5][:, hs], start=True, stop=True, tile_position=(0, r + 32),
                )
        epilogue(coffT_b[0:56, :], psB[0:56, :], bias_sb[0:56, 13:14], False)
        epilogue(coffT_b[64:120, :], psB[64:120, :], bias_sb[64:120, 13:14], False)

        # ---- main GEMM, b-tiles in pairs: per 1024-wide N-chunk, A-passes
        # (K=128) for both b-tiles, then the two K=56 B-passes CONCURRENTLY
        # in PE row groups 0 / 64, then per-chunk evacuation (2^-13 descale,
        # f16) on ACT (tile i) and DVE (tile j) in parallel.
        for p in range(NB // 2):
            bti, btj = 2 * p, 2 * p + 1
            bsl_i = slice(bti * 128, bti * 128 + 128)
            bsl_j = slice(btj * 128, btj * 128 + 128)
            os_i = outp.tile([128, VC3], f16, tag="ostrip", name=f"ostrip_{bti}")
            os_j = outp.tile([128, VC3], f16, tag="ostrip", name=f"ostrip_{btj}")
            for g in range(3):
                g0, g1 = PAIR_BOUNDS[g], PAIR_BOUNDS[g + 1]
                ti = psum.tile([128, 1024], f32, tag="ps", name=f"psm_{p}_{g}_i")
                tj = psum.tile([128, 1024], f32, tag="ps", name=f"psm_{p}_{g}_j")
                subs = [
                    (slice(n0 - g0, n1 - g0), slice(n0, n1))
                    for n0, n1 in zip(NT_BOUNDS, NT_BOUNDS[1:])
                    if g0 <= n0 < g1
                ]
                for ps, bsl in ((ti, bsl_i), (tj, bsl_j)):
                    for ssl, nsl in subs:
                        nc.tensor.matmul(
                            ps[:, ssl], lhsT=coffT_a[:, bsl], rhs=bfm_a[:, nsl],
                            start=True, stop=False,
                        )
                for ssl, nsl in subs:
                    nc.tensor.matmul(
                        ti[:, ssl], lhsT=coffT_b[0:56, bsl_i],
                        rhs=bfm_b[0:56, nsl], start=False, stop=True,
                        tile_position=(0, 0),
                    )
                    nc.tensor.matmul(
                        tj[:, ssl], lhsT=coffT_b[64:120, bsl_j],
                        rhs=bfm_b[64:120, nsl], start=False, stop=True,
                        tile_position=(64, 0),
                    )
                nc.scalar.activation(
                    os_i[:, g0:g1], ti[:, 0 : g1 - g0], AF.Copy, scale=DESCALE
                )
                nc.vector.tensor_scalar(
                    out=os_j[:, g0:g1], in0=tj[:, 0 : g1 - g0], scalar1=DESCALE,
                    scalar2=None, op0=ALU.mult,
                )
                if p == NB // 2 - 1:
                    # last pair: store per chunk so the final transfer is small
                    nc.sync.dma_start(out=res[bsl_i, g0:g1], in_=os_i[:, g0:g1])
                    nc.sync.dma_start(out=res[bsl_j, g0:g1], in_=os_j[:, g0:g1])
            if p < NB // 2 - 1:
                # full-row stores: 5184B HBM segments move faster than the
                # 2048B segments of column-piece stores
                nc.sync.dma_start(out=res[bsl_i, :], in_=os_i[:])
                nc.sync.dma_start(out=res[bsl_j, :], in_=os_j[:])

    nc.finalize()
    return nc


def _pack_host(pose, basis, mask, w1, b1, w2, b2, w3, b3):
    pose_t = pose[:, 1:].reshape(BATCH, 207).T.astype(np.float16)  # [207, B]
    pose_mega = np.zeros((128, 3 * BATCH), np.float16)
    for c, (js, je) in enumerate(CHUNKS):
        K = 9 * (je - js)
        p, hi = divmod(c, 2)
        r0 = 64 if hi else 0
        pose_mega[r0 : r0 + K, p * BATCH : (p + 1) * BATCH] = (
            pose_t[9 * js : 9 * js + K]
        )

    # bfm rows (j, k) scaled by 2^13, cols (v, c) padded to VPAD.
    bfm = np.zeros((N_JOINT * BPJ, VPAD * 3), np.float16)
    prod = (basis[:, None, :, :] * mask[:, :, None, None] * BSCALE)  # (V, J, K, 3)
    bfm[:, : N_VERT * 3] = (
        prod.transpose(1, 2, 0, 3).reshape(N_JOINT * BPJ, N_VERT * 3)
    ).astype(np.float16)

    w_all = np.zeros((128, W_COLS), np.float16)
    bias_all = np.zeros((128, BIAS_COLS), np.float32)
    eye9 = np.eye(3, dtype=np.float64).reshape(-1)
    for c, ((js, je), o1, o2, o3) in enumerate(zip(CHUNKS, W1_OFF, W2_OFF, W3_OFF)):
        r1 = 64 if c % 2 else 0  # odd chunks' W1 blocks live at PE rows 64+
        for t, j in enumerate(range(js, je)):
            w_all[r1 + t * 9 : r1 + (t + 1) * 9, o1 + t * 18 : o1 + (t + 1) * 18] = w1[j]
            w_all[t * 18 : (t + 1) * 18, o2 + t * 32 : o2 + (t + 1) * 32] = w2[j]
            w_all[t * 32 : (t + 1) * 32, o3 + t * 8 : o3 + (t + 1) * 8] = w3[j]
    b1f = b1.astype(np.float64) - np.einsum("i,jio->jo", eye9, w1.astype(np.float64))
    for c, (js, je) in enumerate(CHUNKS):
        nj = je - js
        bias_all[0 : 18 * nj, c] = b1f[js:je].reshape(-1).astype(np.float32)
        bias_all[0 : 32 * nj, 6 + c] = b2[js:je].reshape(-1)
    bias_all[0:128, 12] = b3[0:16].reshape(-1)  # chunks 0-3 stacked (4*32)
    bias_all[0:56, 13] = b3[16:23].reshape(-1)  # chunks 4,5 stacked (32+24)
    bias_all[64:120, 13] = b3[16:23].reshape(-1)  # dup for PE row group 64

    mega = np.concatenate([w_all, pose_mega], axis=1)
    return mega, bfm, w_all, bias_all


def _dup_b(x):
    out = np.zeros((128, x.shape[1]), np.float16)
    out[0:56] = x
    out[64:120] = x
    return np.ascontiguousarray(out)


def _in_maps(pose, basis, mask, w1, b1, w2, b2, w3, b3):
    mega, bfm, w_all, bias_all = _pack_host(
        np.asarray(pose, np.float32),
        np.asarray(basis, np.float32),
        np.asarray(mask, np.float32),
        np.asarray(w1, np.float32),
        np.asarray(b1, np.float32),
        np.asarray(w2, np.float32),
        np.asarray(b2, np.float32),
        np.asarray(w3, np.float32),
        np.asarray(b3, np.float32),
    )
    maps = []
    for i in range(8):
        c0 = i * VC3
        maps.append(
            {
                "mega": mega,
                "bfm_a": np.ascontiguousarray(bfm[0:128, c0 : c0 + VC3]),
                "bfm_b": _dup_b(bfm[128:184, c0 : c0 + VC3]),
                "bias_all": bias_all,
            }
        )
    return maps


def kernel(pose, basis, mask, w1, b1, w2, b2, w3, b3):
    from concourse.bass_utils import run_bass_kernel_spmd

    if "nc" not in _CACHED:
        _CACHED["nc"] = _build_nc()
    nc = _CACHED["nc"]

    maps = _in_maps(pose, basis, mask, w1, b1, w2, b2, w3, b3)
    r = run_bass_kernel_spmd(nc, maps, core_ids=list(range(8)))
    out = np.concatenate(
        [m["res"].astype(np.float32).reshape(BATCH, VC, 3) for m in r.results],
        axis=1,
    )
    return np.ascontiguousarray(out[:, :N_VERT, :])
